# revision 40
# baseline (speedup 1.0000x reference)
"""Multi-head attention (B=2, S=2048, D=1024, H=16) on 8 Trainium2 NeuronCores.

Sharding: data-parallel on batch, tensor-parallel on heads.
Core c handles batch b = c // 4 and heads [4*(c%4), 4*(c%4)+4).
Each core computes its 4 heads' attention + its partial Wo projection;
the host sums the 4 partial [S, D] outputs per batch (the TP all-reduce).

Device-side layout (all bf16 storage, fp32 PSUM accumulation):
- Host pre-transposes query/context to [D, S]; all contractions land on
  SBUF partitions with no input transposes.
- Scores are computed transposed ([c, q]) in [128, 2, 512] PSUM tiles
  (both heads of a pair share one exp).
- PV is FLIPPED: the exp'd probabilities PT are the STATIONARY operand
  ([128, 128] slices; ldweights are engine-free) and vaug ([c, 65],
  ones column = softmax denominator) moves -> po output [q=128, 65].
  Matmul cost is output-free-size only, so this halves PV's PE time
  vs the [65, 512] orientation, and the denominator lands in the same
  partition as its rows.
- Epilogue per (qsub, head): strided reciprocal of the denominator
  column + one per-partition tensor_scalar multiply (PSUM -> bf16
  SBUF), then a PE transpose (via identity) stacks the two heads of a
  pair into outT [hd, q] tiles for the Wo matmul.
- PSUM accumulation groups: start_tensor_calc zeroes the whole 2KB
  bank, so each bank carries exactly ONE group: only the bank's first
  matmul starts, only its last stops; per-byte pending-zero handles
  first-write zeroing of each (qsub, head) sub-region.
- PSUM budget (8 banks): scores 2 bufs x [128,2,512] = 4, po 2 x
  [128,4,65] = 2, shared proj/V/wo/transpose ring = 2.
- Block order: ALL pair-0 q-chunks, then all pair-1. After the
  projection-heavy first block the ACT exp stream (133us) is the
  kernel's floor; pair-0-first keeps later blocks ACT-paced. V
  projection is split per pair so pair-1's share runs as fillers in
  later pair-0 blocks.
- Fillers are spread ~1 step (2 matmuls, 427ns) per c-tile slot: the
  per-ct PE budget under the 1038ns exp is ~394ns, and overflowing a
  slot costs a psS-ring ping-pong. PV matmuls are emitted AFTER the
  slot's fillers so they don't camp in the 4-deep wait queue.
- qT/kT live as per-512-column-chunk tiles (hazard granularity), and
  the tail spreads the last Wo accumulators over psS/psA/psW banks
  with its four out-DMAs on different issue queues.
"""

import numpy as np
import ml_dtypes

import concourse.bacc as bacc
import concourse.mybir as mybir
from concourse.tile import TileContext
from concourse.bass_utils import run_bass_kernel_spmd

BF16 = mybir.dt.bfloat16
F32 = mybir.dt.float32

B, S, D, H = 2, 2048, 1024, 16
SPH = D // H          # 64
NH = 4                # heads per core
P = 128               # SBUF partitions
DC = D // P           # 8 d-chunks
CT = S // P           # 16 c-tiles
NEG_INF = -1e9

_NC_CACHE = {}
# debug switch: emit all filler generators between blocks instead of
# interleaved into them (baseline-style scheduling)
NOFILL = False


def _build(masked: bool):
    nc = bacc.Bacc("TRN2", target_bir_lowering=False, debug=False, num_devices=8)

    qt_d = nc.declare_dram_parameter("qt", [D, S], BF16, isOutput=False)
    ct_d = nc.declare_dram_parameter("ctx", [D, S], BF16, isOutput=False)
    wq_d = nc.declare_dram_parameter("wq", [D, NH * SPH], BF16, isOutput=False)
    wk_d = nc.declare_dram_parameter("wk", [D, NH * SPH], BF16, isOutput=False)
    wv_d = nc.declare_dram_parameter("wv", [D, NH * SPH], BF16, isOutput=False)
    wo_d = nc.declare_dram_parameter("wo", [NH * SPH, D], BF16, isOutput=False)
    id_d = nc.declare_dram_parameter("ident", [P, P], BF16, isOutput=False)
    if masked:
        mk_d = nc.declare_dram_parameter("maskT", [S, S], BF16, isOutput=False)
    out_d = nc.declare_dram_parameter("out", [S, D], BF16, isOutput=True)

    with TileContext(nc) as tc:
        with (
            tc.tile_pool(name="const", bufs=1) as const,
            tc.tile_pool(name="work", bufs=1) as work,
            tc.tile_pool(name="pt", bufs=4) as ptp,
            tc.tile_pool(name="outp", bufs=4) as outp,
            tc.tile_pool(name="epi", bufs=4) as epi,
            tc.tile_pool(name="psS", bufs=2, space="PSUM") as psS,
            tc.tile_pool(name="psA", bufs=1, space="PSUM") as psA,
            tc.tile_pool(name="psW", bufs=2, space="PSUM") as psW,
        ):
            # ---- staged inputs ------------------------------------------
            wq_sb = const.tile([P, DC, NH * SPH], BF16)
            wk_sb = const.tile([P, DC, NH * SPH], BF16)
            wv_sb = const.tile([P, DC, NH * SPH], BF16)
            # wo rows are (h, s); head pair t = h//2 packs two heads into the
            # partition dim (head h%2==0 -> partitions 0-63, ==1 -> 64-127).
            wo_sb = const.tile([P, 2, D], BF16)
            id_sb = const.tile([P, P], BF16)
            qt_sb = const.tile([P, DC, S], BF16)
            ct_sb = const.tile([P, DC, S], BF16)
            qt_r = qt_d[:, :].rearrange("(c p) q -> p c q", p=P)
            ct_r = ct_d[:, :].rearrange("(c p) q -> p c q", p=P)

            # DMA order tuned so the first k/q projection pieces start early
            # and each later dependency lands just ahead of its use. The
            # first wk/ctx pieces are dc-split so k-proj's dc 0-3 matmuls
            # start while dc 4-7 data is still in flight. ctx chunks 1-3
            # come before qt 1-3: they gate block (0,0)'s exp stream.
            wk_r = wk_d[:, :].rearrange("(c p) n -> p c n", p=P)
            nc.sync.dma_start(out=wk_sb[:, 0:4, :], in_=wk_r[:, 0:4, :])
            nc.sync.dma_start(out=ct_sb[:, 0:4, 0:256], in_=ct_r[:, 0:4, 0:256])
            nc.sync.dma_start(out=wk_sb[:, 4:8, :], in_=wk_r[:, 4:8, :])
            nc.sync.dma_start(out=ct_sb[:, 4:8, 0:256], in_=ct_r[:, 4:8, 0:256])
            nc.sync.dma_start(out=wq_sb, in_=wq_d[:, :].rearrange("(c p) n -> p c n", p=P))
            nc.sync.dma_start(out=qt_sb[:, :, 0:256], in_=qt_r[:, :, 0:256])
            nc.sync.dma_start(out=qt_sb[:, :, 256:512], in_=qt_r[:, :, 256:512])
            nc.sync.dma_start(out=ct_sb[:, :, 256:512], in_=ct_r[:, :, 256:512])
            nc.sync.dma_start(out=wv_sb, in_=wv_d[:, :].rearrange("(c p) n -> p c n", p=P))
            # ident (first used ~25us in) rides the Pool queue, emitted after
            # the latency-critical input pieces so its transfer doesn't cut
            # into the serialized DMA stream
            nc.gpsimd.dma_start(out=id_sb, in_=id_d[:, :])
            for i in range(1, 4):
                nc.sync.dma_start(out=ct_sb[:, :, 512 * i:512 * (i + 1)],
                                  in_=ct_r[:, :, 512 * i:512 * (i + 1)])
            for i in range(1, 4):
                nc.sync.dma_start(out=qt_sb[:, :, 512 * i:512 * (i + 1)],
                                  in_=qt_r[:, :, 512 * i:512 * (i + 1)])
            nc.sync.dma_start(out=wo_sb, in_=wo_d[:, :].rearrange("(t x) d -> x t d", x=P))

            # ---- persistent SBUF tensors --------------------------------
            # qT/kT are split per 512-column CHUNK: tile-granular hazard
            # tracking would otherwise serialize a block's scores behind any
            # same-pair projection filler writing another chunk of the tile
            qTc = [[work.tile([P, 512], BF16, tag=f"qT{p}{c}", name=f"qT{p}{c}")
                    for c in range(4)] for p in range(2)]
            kTc = [[work.tile([P, 512], BF16, tag=f"kT{p}{c}", name=f"kT{p}{c}")
                    for c in range(4)] for p in range(2)]
            # vaug / outT split into small tiles: Tile hazard tracking is
            # per-tile, shared big tiles create false deps between writers
            # and concurrent readers. vaug is per-PAIR so pair-1's V
            # projection can run as fillers in later blocks.
            vaug = [[work.tile([P, 2, 80], BF16, tag=f"va{p}{ct}",
                               name=f"va{p}{ct}") for ct in range(CT)]
                    for p in range(2)]
            for p in range(2):
                for ct in range(CT):
                    nc.vector.memset(vaug[p][ct][:, :, SPH:SPH + 1], 1.0)
            # outT[qc4][p][qsub]: [pair-head rows (a*64+s), 128 q cols]
            outT = [[[work.tile([P, P], BF16, tag=f"oT{qc4}{p}{qs}",
                                name=f"oT{qc4}{p}{qs}") for qs in range(4)]
                     for p in range(2)]
                    for qc4 in range(4)]

            # ---- filler generators: each yield = ~2 PE matmuls ----------
            def gen_proj(which, p, qc4, csplit=None):
                """q/k projection chunk -> qTc/kTc[p][qc4].
                csplit=(lo,hi) restricts to a column sub-range (lead-in)."""
                src_sb = wk_sb if which == "k" else wq_sb
                x_sb = ct_sb if which == "k" else qt_sb
                dst = (kTc if which == "k" else qTc)[p][qc4]
                lo, hi = csplit if csplit else (512 * qc4, 512 * (qc4 + 1))
                w = hi - lo
                ps = psW.tile([P, w], F32, tag="W", name=f"ps{which}{p}{qc4}")
                for dc in range(DC):
                    nc.tensor.matmul(
                        ps[:, :],
                        src_sb[:, dc, P * p:P * (p + 1)],
                        x_sb[:, dc, lo:hi],
                        start=(dc == 0), stop=(dc == DC - 1),
                    )
                    if dc % 2 == 1 and dc != DC - 1:
                        yield
                # copy on DVE, not ACT: an ACT copy interleaved between exp's
                # would delay the exp stream that paces the attention loop
                nc.vector.tensor_copy(dst[:, lo - 512 * qc4:hi - 512 * qc4],
                                      ps[:, :])
                yield

            def gen_vproj(p, ct):
                """V projection (one pair's 2 heads) for one c-tile."""
                psv = psW.tile([P, 2 * SPH], F32, tag="W", name=f"psv{p}{ct}")
                for dc in range(DC):
                    nc.tensor.matmul(
                        psv[:, :],
                        ct_sb[:, dc, P * ct:P * (ct + 1)],
                        wv_sb[:, dc, P * p:P * (p + 1)],
                        start=(dc == 0), stop=(dc == DC - 1),
                    )
                nc.vector.tensor_copy(
                    vaug[p][ct][:, :, 0:SPH],
                    psv[:, :].rearrange("p (h s) -> p h s", h=2),
                )
                yield

            def gen_wo(qc4):
                """Output projection for q-chunk qc4: per (qt4, dh) one
                [128,512] accumulator over the 2 pairs."""
                for qt4 in range(4):
                    qt = 4 * qc4 + qt4
                    osb = outp.tile([P, D], BF16, tag="osb", name="osb")
                    for dh in range(2):
                        wx = psW.tile([P, 512], F32, tag="W", name="wx")
                        for p2 in range(2):
                            nc.tensor.matmul(
                                wx[:, :],
                                outT[qc4][p2][qt4][:, :],
                                wo_sb[:, p2, 512 * dh:512 * (dh + 1)],
                                start=(p2 == 0), stop=(p2 == 1))
                        nc.vector.tensor_copy(osb[:, 512 * dh:512 * (dh + 1)], wx)
                        if dh == 0:
                            yield
                    # DMA emitted before the final yield: a generator pumped an
                    # exact number of steps never runs code after its last yield
                    eng = nc.gpsimd if qt4 % 2 == 0 else nc.sync
                    eng.dma_start(out=out_d[P * qt:P * (qt + 1), :], in_=osb)
                    yield

            # ---- epilogue: normalize po rows by the denominator column,
            # transpose into outT. 7 steps, software-pipelined: step k does
            # the ts-muls for qsub k and the transposes/outT copy for qsub
            # k-1, so the PE transposes always consume slot-old DVE results
            # instead of stalling on same-slot ts-muls.
            def gen_epi(poAB, qc4, p):
                stg, rc = [], []
                for a in range(2):
                    # stage po to SBUF eagerly so the po bank recycles before
                    # the next block's first PV accumulation
                    s = epi.tile([P, 4, 65], F32, tag="stg", name="stg")
                    nc.vector.tensor_copy(s, poAB[a][:, :, :])
                    r = epi.tile([P, 4], F32, tag="rc", name="rc")
                    nc.vector.reciprocal(r, s[:, :, SPH:SPH + 1])
                    stg.append(s)
                    rc.append(r)
                    yield
                pns = [None] * 4
                for k in range(5):
                    if k < 4:
                        pq = []
                        for a in range(2):
                            pn = epi.tile([P, SPH], BF16, tag="pn", name="pn")
                            nc.vector.tensor_scalar_mul(
                                pn, stg[a][:, k, 0:SPH], rc[a][:, k:k + 1])
                            pq.append(pn)
                        pns[k] = pq
                    if k >= 1:
                        qs = k - 1
                        trp = psW.tile([P, P], BF16, tag="W", name="trp")
                        for a in range(2):
                            nc.tensor.matmul(
                                trp[SPH * a:SPH * (a + 1), :], pns[qs][a],
                                id_sb, is_transpose=True)
                        nc.vector.tensor_copy(outT[qc4][p][qs][:, :], trp)
                    yield

            # ---- attention block: two heads of a pair interleaved -------
            # Heads a=0/1 live at partition bases 0/64 in qTp/kTp; scores for
            # both go into one [128,2,512] PSUM tile so a single exp covers
            # both heads. slots: per-ct filler generators (~2 steps each).
            def attn_block(qc4, p, slots):
                q0 = 512 * qc4
                if NOFILL:
                    seen = []
                    for gens in slots.values():
                        for gen, _ in gens:
                            if all(gen is not g for g in seen):
                                seen.append(gen)
                    for gen in seen:
                        for _ in gen:
                            pass
                poAB = [psA.tile([P, 4, 65], F32, tag=f"A{a}", name=f"po{a}")
                        for a in range(2)]
                pend = None
                for ct in range(CT + 1):
                    if ct < CT:
                        Sp = psS.tile([P, 2, 512], F32, tag="S", name="Sp")
                        for a in range(2):
                            lo, hi = SPH * a, SPH * (a + 1)
                            nc.tensor.matmul(
                                Sp[:, a, :],
                                kTc[p][ct // 4][lo:hi,
                                                P * (ct % 4):P * (ct % 4 + 1)],
                                qTc[p][qc4][lo:hi, :],
                                start=True, stop=True)
                        if masked:
                            mk = ptp.tile([P, 512], BF16, tag="mk", name="mk")
                            nc.sync.dma_start(
                                out=mk,
                                in_=mk_d[P * ct:P * (ct + 1), q0:q0 + 512])
                            for a in range(2):
                                nc.vector.tensor_add(Sp[:, a, :], Sp[:, a, :], mk)
                    if ct >= 1:
                        pct = ct - 1
                        PT = ptp.tile([P, 2, 512], BF16, tag="PT", name="PT")
                        nc.scalar.activation(
                            PT[:, :, :], pend[:, :, :],
                            mybir.ActivationFunctionType.Exp)
                    if ct < CT:
                        pend = Sp
                        if not NOFILL:
                            for gen, nsteps in slots.get(ct, ()):
                                for _ in range(nsteps):
                                    try:
                                        next(gen)
                                    except StopIteration:
                                        break
                    if ct >= 1:
                        # flipped PV: PT slices stationary, vaug moving.
                        # One accumulation group per po BANK: start only on
                        # the bank's first matmul, stop only on its last.
                        # Emitted AFTER the fillers: PE reaches these just as
                        # exp(ct-1) completes -- emitting them earlier parks
                        # them in the 4-deep wait queue, which blocks the
                        # younger fillers/scores and adds ~140ns per ct.
                        for a in range(2):
                            for qs in range(4):
                                nc.tensor.matmul(
                                    poAB[a][:, qs, :],
                                    PT[:, a, P * qs:P * (qs + 1)],
                                    vaug[p][pct][:, a, 0:SPH + 1],
                                    start=(pct == 0 and qs == 0),
                                    stop=(pct == CT - 1 and qs == 3))
                return poAB

            def drain(gen):
                for _ in gen:
                    pass

            # ---- schedule: all pair-0 blocks, then all pair-1 blocks ----
            # After the projection-heavy first block, the ACT exp stream
            # (133us total) is the kernel's floor; pair-0-first keeps every
            # later block ACT-paced with evenly spread PE fillers.

            # lead-in: race the input DMA (k/q chunk 0 in arrival order)
            drain(gen_proj("k", 0, 0, csplit=(0, 256)))
            drain(gen_proj("q", 0, 0, csplit=(0, 256)))
            drain(gen_proj("q", 0, 0, csplit=(256, 512)))
            drain(gen_proj("k", 0, 0, csplit=(256, 512)))

            # block (0,0): pair-0 V projections (1 step per ct, just in
            # time for PV) + remaining kTp[0] chunks racing the ctx DMA.
            vg0 = {ct: gen_vproj(0, ct) for ct in range(CT)}
            vg1 = {ct: gen_vproj(1, ct) for ct in range(CT)}
            g_k01 = gen_proj("k", 0, 1)
            g_k02 = gen_proj("k", 0, 2)
            g_k03 = gen_proj("k", 0, 3)
            g_q01 = gen_proj("q", 0, 1)
            po00 = attn_block(0, 0, {
                0: [(vg0[0], 1)], 1: [(vg0[1], 1), (g_k01, 2)],
                2: [(vg0[2], 1), (g_k01, 2)], 3: [(vg0[3], 1)],
                4: [(vg0[4], 1)], 5: [(vg0[5], 1), (g_k02, 2)],
                6: [(vg0[6], 1), (g_k02, 2)], 7: [(vg0[7], 1)],
                8: [(vg0[8], 1)], 9: [(vg0[9], 1), (g_k03, 2)],
                10: [(vg0[10], 1), (g_k03, 2)], 11: [(vg0[11], 1)],
                12: [(vg0[12], 1), (g_q01, 2)],
                13: [(vg0[13], 1), (g_q01, 2)],
                14: [(vg0[14], 1)], 15: [(vg0[15], 1)],
            })

            # pair-0 blocks carry pair-1's V projection and the q1x chunks
            # (opposite-pair tiles: no hazard against their own scores)
            # one filler step (~427ns) per slot: a 2-step slot overflows the
            # per-ct ACT budget and each overflow costs a psS-ring ping-pong
            g_epi = gen_epi(po00, 0, 0)
            g_q02 = gen_proj("q", 0, 2)
            g_q11 = gen_proj("q", 1, 1)
            # epilogue qs-steps spread one per slot (0-4): each transpose's
            # psW slot frees before the next psW-using filler allocates
            po10 = attn_block(1, 0, {
                0: [(g_epi, 2)], 1: [(g_epi, 1)], 2: [(g_epi, 1), (vg1[0], 1)],
                3: [(g_epi, 1), (g_q02, 1)], 4: [(g_epi, 1), (vg1[1], 1)],
                5: [(g_epi, 1)],
                6: [(vg1[2], 1), (g_q02, 1)], 7: [(g_q02, 1)], 8: [(vg1[3], 1)],
                9: [(g_q02, 1)], 10: [(g_q11, 1)], 11: [(vg1[4], 1)],
                12: [(g_q11, 1)], 13: [(g_q11, 1)], 14: [(g_q11, 1)],
                15: [(vg1[5], 1)],
            })

            g_epi = gen_epi(po10, 1, 0)
            g_q03 = gen_proj("q", 0, 3)
            g_q12 = gen_proj("q", 1, 2)
            po20 = attn_block(2, 0, {
                0: [(g_epi, 2)], 1: [(g_epi, 1)], 2: [(g_epi, 1), (vg1[6], 1)],
                3: [(g_epi, 1), (g_q03, 1)], 4: [(g_epi, 1), (vg1[7], 1)],
                5: [(g_epi, 1)],
                6: [(vg1[8], 1), (g_q03, 1)], 7: [(g_q03, 1)], 8: [(vg1[9], 1)],
                9: [(g_q03, 1)], 10: [(g_q12, 1)], 11: [(vg1[10], 1)],
                12: [(g_q12, 1)], 13: [(g_q12, 1)], 14: [(g_q12, 1)],
            })

            g_epi = gen_epi(po20, 2, 0)
            g_q10 = gen_proj("q", 1, 0)
            g_k10 = gen_proj("k", 1, 0)
            po30 = attn_block(3, 0, {
                0: [(g_epi, 2)], 1: [(g_epi, 1)], 2: [(g_epi, 1), (vg1[11], 1)],
                3: [(g_epi, 1), (g_q10, 1)], 4: [(g_epi, 1), (vg1[12], 1)],
                5: [(g_epi, 1)],
                6: [(vg1[13], 1), (g_q10, 1)], 7: [(g_q10, 1)], 8: [(vg1[14], 1)],
                9: [(g_q10, 1)], 10: [(g_k10, 1)], 11: [(vg1[15], 1)],
                12: [(g_k10, 1)], 13: [(g_k10, 1)], 14: [(g_k10, 1)],
            })

            # pair-1 blocks: k1x chunks (same-pair, subtile deps), then one
            # wo per block chasing its own epilogue. k11 must land by slot 2
            # (scores(4) read it), so it alone gets 2-step slots.
            g_epi = gen_epi(po30, 3, 0)
            g_k11 = gen_proj("k", 1, 1)
            g_k12 = gen_proj("k", 1, 2)
            g_k13 = gen_proj("k", 1, 3)
            po01 = attn_block(0, 1, {
                0: [(g_epi, 2), (g_k11, 2)],
                1: [(g_epi, 1), (g_k11, 2)], 2: [(g_epi, 1), (g_k12, 1)],
                3: [(g_epi, 1), (g_k12, 1)], 4: [(g_epi, 1), (g_k12, 1)],
                5: [(g_epi, 1), (g_k12, 1)],
                6: [(g_k13, 1)], 7: [(g_k13, 1)], 8: [(g_k13, 1)],
                9: [(g_k13, 1)],
            })

            g_epi = gen_epi(po01, 0, 1)
            g_wo0 = gen_wo(0)
            g_q13 = gen_proj("q", 1, 3)
            po11 = attn_block(1, 1, {
                0: [(g_epi, 2)], 1: [(g_epi, 1)], 2: [(g_epi, 1)],
                3: [(g_epi, 1), (g_q13, 1)], 4: [(g_epi, 1), (g_q13, 1)],
                5: [(g_epi, 1), (g_q13, 1)],
                6: [(g_q13, 1)],
                7: [(g_wo0, 1)], 8: [(g_wo0, 1)], 9: [(g_wo0, 1)],
                10: [(g_wo0, 1)], 11: [(g_wo0, 1)], 12: [(g_wo0, 1)],
                13: [(g_wo0, 1)], 14: [(g_wo0, 1)],
            })

            g_epi = gen_epi(po11, 1, 1)
            g_wo1 = gen_wo(1)
            po21 = attn_block(2, 1, {
                0: [(g_epi, 2)], 1: [(g_epi, 1)], 2: [(g_epi, 1)],
                3: [(g_epi, 1)], 4: [(g_epi, 1)], 5: [(g_epi, 1)],
                7: [(g_wo1, 1)], 8: [(g_wo1, 1)], 9: [(g_wo1, 1)],
                10: [(g_wo1, 1)], 11: [(g_wo1, 1)], 12: [(g_wo1, 1)],
                13: [(g_wo1, 1)], 14: [(g_wo1, 1)],
            })

            g_epi = gen_epi(po21, 2, 1)
            g_wo2 = gen_wo(2)
            po31 = attn_block(3, 1, {
                0: [(g_epi, 2)], 1: [(g_epi, 1)], 2: [(g_epi, 1)],
                3: [(g_epi, 1)], 4: [(g_epi, 1)], 5: [(g_epi, 1)],
                7: [(g_wo2, 1)], 8: [(g_wo2, 1)], 9: [(g_wo2, 1)],
                10: [(g_wo2, 1)], 11: [(g_wo2, 1)], 12: [(g_wo2, 1)],
                13: [(g_wo2, 1)], 14: [(g_wo2, 1)],
            })

            # ---- tail: last epilogue + wo(3) ----------------------------
            # Latency-bound chain off the last exp: read po from PSUM
            # directly (no staging), run all four qsubs' normalize/transpose
            # chains first, then the wo matmuls + copies (DVE/ACT split) +
            # out-DMA halves as each half lands.
            rc = []
            for a in range(2):
                r = epi.tile([P, 4], F32, tag="rc", name=f"trc{a}")
                nc.vector.reciprocal(r, po31[a][:, :, SPH:SPH + 1])
                rc.append(r)
            for qs in range(4):
                trp = psW.tile([P, P], BF16, tag="W", name="trp")
                for a in range(2):
                    pn = epi.tile([P, SPH], BF16, tag="pn", name="pn")
                    # a=0 normalize on DVE, a=1 on ACT: halves the serial
                    # epilogue chain feeding the transposes
                    if a == 0:
                        nc.vector.tensor_scalar_mul(
                            pn, po31[a][:, qs, 0:SPH], rc[a][:, qs:qs + 1])
                    else:
                        nc.scalar.mul(pn, po31[a][:, qs, 0:SPH],
                                      rc[a][:, qs:qs + 1])
                    nc.tensor.matmul(
                        trp[SPH * a:SPH * (a + 1), :], pn, id_sb,
                        is_transpose=True)
                # outT copies split ACT/DVE
                if qs % 2 == 0:
                    nc.scalar.copy(outT[3][1][qs][:, :], trp)
                else:
                    nc.vector.tensor_copy(outT[3][1][qs][:, :], trp)
            osbt = [outp.tile([P, D], BF16, tag="osb", name=f"osbt{q}")
                    for q in range(4)]
            # each qs gets its own PSUM banks (psS slots, the dead po banks,
            # and the psW ring) so no wo matmul waits on an earlier qs's
            # copies. All pair-0 (start) matmuls are emitted first: the
            # pair-1 (stop) matmuls park in the 4-deep wait queue and would
            # otherwise block younger pair-0 work. qs3 takes the po banks
            # (drained by the ts-muls) -- earlier than the psW ring, which
            # still cycles the transposes.
            wxq = [
                psS.tile([P, 2, 512], F32, tag="S", name="twx0"),
                psS.tile([P, 2, 512], F32, tag="S", name="twx1"),
                [psW.tile([P, 512], F32, tag="W", name="twx2a"),
                 psW.tile([P, 512], F32, tag="W", name="twx2b")],
                [psA.tile([P, 512], F32, tag="A0", name="twx3a"),
                 psA.tile([P, 512], F32, tag="A1", name="twx3b")],
            ]

            def wxv(qs, dh):
                return wxq[qs][:, dh, :] if qs < 2 else wxq[qs][dh][:, :]

            def wo_p0(qs):
                for dh in range(2):
                    nc.tensor.matmul(
                        wxv(qs, dh), outT[3][0][qs][:, :],
                        wo_sb[:, 0, 512 * dh:512 * (dh + 1)],
                        start=True, stop=False, skip_group_check=True)

            def wo_p1(qs):
                qt = 4 * 3 + qs
                for dh in range(2):
                    nc.tensor.matmul(
                        wxv(qs, dh), outT[3][1][qs][:, :],
                        wo_sb[:, 1, 512 * dh:512 * (dh + 1)],
                        start=False, stop=True, skip_group_check=True)
                    dst = osbt[qs][:, 512 * dh:512 * (dh + 1)]
                    # dh0 on ACT, dh1 on DVE: parallel drains
                    if dh == 0:
                        nc.scalar.copy(dst, wxv(qs, dh))
                    else:
                        nc.vector.tensor_copy(dst, wxv(qs, dh))
                # one full-tile DMA per qs, spread across the three DMA-
                # capable queues: the per-queue wait+issue pipeline (~1.2us
                # per DMA, serialized) would otherwise gate the last transfer
                eng = [nc.gpsimd, nc.scalar, nc.gpsimd, nc.sync][qs]
                eng.dma_start(out=out_d[P * qt:P * (qt + 1), :], in_=osbt[qs])

            # qs2/qs3's pair-0 matmuls wait on the po-bank / psW-ring drain;
            # emitting them before qs0/qs1's pair-1 work would fill the
            # 4-deep wait queue and block it
            wo_p0(0)
            wo_p0(1)
            wo_p1(0)
            wo_p1(1)
            wo_p0(3)
            wo_p0(2)
            wo_p1(2)
            wo_p1(3)

    nc.compile()
    return nc


def _get_nc(masked: bool):
    if masked not in _NC_CACHE:
        _NC_CACHE[masked] = _build(masked)
    return _NC_CACHE[masked]


def kernel(query, context, attention_mask, Wq, Wk, Wv, Wo, **_unused):
    query = np.asarray(query, dtype=np.float32)
    context = np.asarray(context, dtype=np.float32)
    attention_mask = np.asarray(attention_mask, dtype=np.float32)
    Wq = np.asarray(Wq, dtype=np.float32)
    Wk = np.asarray(Wk, dtype=np.float32)
    Wv = np.asarray(Wv, dtype=np.float32)
    Wo = np.asarray(Wo, dtype=np.float32)

    masked = bool(np.any(attention_mask))
    nc = _get_nc(masked)

    bf = ml_dtypes.bfloat16
    # fold the 1/sqrt(SPH) score scale into Wq
    wq_s = (Wq * (SPH ** -0.5)).astype(bf)
    wk_s = Wk.astype(bf)
    wv_s = Wv.astype(bf)
    wo_s = Wo.astype(bf)
    ident = np.eye(P, dtype=bf)

    qtT = [np.ascontiguousarray(query[b].T).astype(bf) for b in range(B)]
    ctT = [np.ascontiguousarray(context[b].T).astype(bf) for b in range(B)]
    if masked:
        mkT = [np.ascontiguousarray((attention_mask[b, 0] * NEG_INF).T).astype(bf)
               for b in range(B)]

    in_maps = []
    for c in range(8):
        b, g = c // 4, c % 4
        hs = slice(NH * g, NH * (g + 1))
        im = {
            "qt": qtT[b],
            "ctx": ctT[b],
            "wq": np.ascontiguousarray(wq_s[:, hs, :]).reshape(D, NH * SPH),
            "wk": np.ascontiguousarray(wk_s[:, hs, :]).reshape(D, NH * SPH),
            "wv": np.ascontiguousarray(wv_s[:, hs, :]).reshape(D, NH * SPH),
            "wo": np.ascontiguousarray(wo_s[hs]).reshape(NH * SPH, D),
            "ident": ident,
        }
        if masked:
            im["maskT"] = mkT[b]
        in_maps.append(im)

    global _last_in_maps
    _last_in_maps = in_maps
    res = run_bass_kernel_spmd(nc, in_maps, core_ids=list(range(8)))

    out = np.zeros((B, S, D), dtype=np.float32)
    for c in range(8):
        out[c // 4] += res.results[c]["out"].astype(np.float32)
    return out


# revision 41
# speedup vs baseline: 1.0019x; 1.0019x over previous
"""Multi-head attention (B=2, S=2048, D=1024, H=16) on 8 Trainium2 NeuronCores.

Sharding: data-parallel on batch, tensor-parallel on heads.
Core c handles batch b = c // 4 and heads [4*(c%4), 4*(c%4)+4).
Each core computes its 4 heads' attention + its partial Wo projection;
the host sums the 4 partial [S, D] outputs per batch (the TP all-reduce).

Device-side layout (all bf16 storage, fp32 PSUM accumulation):
- Host pre-transposes query/context to [D, S]; all contractions land on
  SBUF partitions with no input transposes.
- Scores are computed transposed ([c, q]) in [128, 2, 512] PSUM tiles
  (both heads of a pair share one exp).
- PV is FLIPPED: the exp'd probabilities PT are the STATIONARY operand
  ([128, 128] slices; ldweights are engine-free) and vaug ([c, 65],
  ones column = softmax denominator) moves -> po output [q=128, 65].
  Matmul cost is output-free-size only, so this halves PV's PE time
  vs the [65, 512] orientation, and the denominator lands in the same
  partition as its rows.
- Epilogue per (qsub, head): strided reciprocal of the denominator
  column + one per-partition tensor_scalar multiply (PSUM -> bf16
  SBUF), then a PE transpose (via identity) stacks the two heads of a
  pair into outT [hd, q] tiles for the Wo matmul.
- PSUM accumulation groups: start_tensor_calc zeroes the whole 2KB
  bank, so each bank carries exactly ONE group: only the bank's first
  matmul starts, only its last stops; per-byte pending-zero handles
  first-write zeroing of each (qsub, head) sub-region.
- PSUM budget (8 banks): scores 2 bufs x [128,2,512] = 4, po 2 x
  [128,4,65] = 2, shared proj/V/wo/transpose ring = 2.
- Block order: ALL pair-0 q-chunks, then all pair-1. After the
  projection-heavy first block the ACT exp stream (133us) is the
  kernel's floor; pair-0-first keeps later blocks ACT-paced. V
  projection is split per pair so pair-1's share runs as fillers in
  later pair-0 blocks.
- Fillers are spread ~1 step (2 matmuls, 427ns) per c-tile slot: the
  per-ct PE budget under the 1038ns exp is ~394ns, and overflowing a
  slot costs a psS-ring ping-pong. PV matmuls are emitted AFTER the
  slot's fillers so they don't camp in the 4-deep wait queue.
- qT/kT live as per-512-column-chunk tiles (hazard granularity), and
  the tail spreads the last Wo accumulators over psS/psA/psW banks
  with its four out-DMAs on different issue queues.
"""

import numpy as np
import ml_dtypes

import concourse.bacc as bacc
import concourse.mybir as mybir
from concourse.tile import TileContext
from concourse.bass_utils import run_bass_kernel_spmd

BF16 = mybir.dt.bfloat16
F32 = mybir.dt.float32

B, S, D, H = 2, 2048, 1024, 16
SPH = D // H          # 64
NH = 4                # heads per core
P = 128               # SBUF partitions
DC = D // P           # 8 d-chunks
CT = S // P           # 16 c-tiles
NEG_INF = -1e9

_NC_CACHE = {}
# debug switch: emit all filler generators between blocks instead of
# interleaved into them (baseline-style scheduling)
NOFILL = False


def _build(masked: bool):
    nc = bacc.Bacc("TRN2", target_bir_lowering=False, debug=False, num_devices=8)

    qt_d = nc.declare_dram_parameter("qt", [D, S], BF16, isOutput=False)
    ct_d = nc.declare_dram_parameter("ctx", [D, S], BF16, isOutput=False)
    wq_d = nc.declare_dram_parameter("wq", [D, NH * SPH], BF16, isOutput=False)
    wk_d = nc.declare_dram_parameter("wk", [D, NH * SPH], BF16, isOutput=False)
    wv_d = nc.declare_dram_parameter("wv", [D, NH * SPH], BF16, isOutput=False)
    wo_d = nc.declare_dram_parameter("wo", [NH * SPH, D], BF16, isOutput=False)
    id_d = nc.declare_dram_parameter("ident", [P, P], BF16, isOutput=False)
    if masked:
        mk_d = nc.declare_dram_parameter("maskT", [S, S], BF16, isOutput=False)
    out_d = nc.declare_dram_parameter("out", [S, D], BF16, isOutput=True)

    with TileContext(nc) as tc:
        with (
            tc.tile_pool(name="const", bufs=1) as const,
            tc.tile_pool(name="work", bufs=1) as work,
            tc.tile_pool(name="pt", bufs=4) as ptp,
            tc.tile_pool(name="outp", bufs=4) as outp,
            tc.tile_pool(name="epi", bufs=4) as epi,
            tc.tile_pool(name="psS", bufs=2, space="PSUM") as psS,
            tc.tile_pool(name="psA", bufs=1, space="PSUM") as psA,
            tc.tile_pool(name="psW", bufs=2, space="PSUM") as psW,
        ):
            # ---- staged inputs ------------------------------------------
            wq_sb = const.tile([P, DC, NH * SPH], BF16)
            wk_sb = const.tile([P, DC, NH * SPH], BF16)
            wv_sb = const.tile([P, DC, NH * SPH], BF16)
            # wo rows are (h, s); head pair t = h//2 packs two heads into the
            # partition dim (head h%2==0 -> partitions 0-63, ==1 -> 64-127).
            wo_sb = const.tile([P, 2, D], BF16)
            id_sb = const.tile([P, P], BF16)
            qt_sb = const.tile([P, DC, S], BF16)
            ct_sb = const.tile([P, DC, S], BF16)
            qt_r = qt_d[:, :].rearrange("(c p) q -> p c q", p=P)
            ct_r = ct_d[:, :].rearrange("(c p) q -> p c q", p=P)

            # DMA order tuned so the first k/q projection pieces start early
            # and each later dependency lands just ahead of its use. The
            # first wk/ctx pieces are dc-split so k-proj's dc 0-3 matmuls
            # start while dc 4-7 data is still in flight. ctx chunks 1-3
            # come before qt 1-3: they gate block (0,0)'s exp stream.
            wk_r = wk_d[:, :].rearrange("(c p) n -> p c n", p=P)
            nc.sync.dma_start(out=wk_sb[:, 0:4, :], in_=wk_r[:, 0:4, :])
            nc.sync.dma_start(out=ct_sb[:, 0:4, 0:256], in_=ct_r[:, 0:4, 0:256])
            nc.sync.dma_start(out=wk_sb[:, 4:8, :], in_=wk_r[:, 4:8, :])
            nc.sync.dma_start(out=ct_sb[:, 4:8, 0:256], in_=ct_r[:, 4:8, 0:256])
            nc.sync.dma_start(out=wq_sb, in_=wq_d[:, :].rearrange("(c p) n -> p c n", p=P))
            nc.sync.dma_start(out=qt_sb[:, :, 0:256], in_=qt_r[:, :, 0:256])
            nc.sync.dma_start(out=qt_sb[:, :, 256:512], in_=qt_r[:, :, 256:512])
            nc.sync.dma_start(out=ct_sb[:, :, 256:512], in_=ct_r[:, :, 256:512])
            nc.sync.dma_start(out=wv_sb, in_=wv_d[:, :].rearrange("(c p) n -> p c n", p=P))
            # ident (first used ~25us in) rides the Pool queue, emitted after
            # the latency-critical input pieces so its transfer doesn't cut
            # into the serialized DMA stream
            nc.gpsimd.dma_start(out=id_sb, in_=id_d[:, :])
            for i in range(1, 4):
                nc.sync.dma_start(out=ct_sb[:, :, 512 * i:512 * (i + 1)],
                                  in_=ct_r[:, :, 512 * i:512 * (i + 1)])
            for i in range(1, 4):
                nc.sync.dma_start(out=qt_sb[:, :, 512 * i:512 * (i + 1)],
                                  in_=qt_r[:, :, 512 * i:512 * (i + 1)])
            nc.sync.dma_start(out=wo_sb, in_=wo_d[:, :].rearrange("(t x) d -> x t d", x=P))

            # ---- persistent SBUF tensors --------------------------------
            # qT/kT are split per 512-column CHUNK: tile-granular hazard
            # tracking would otherwise serialize a block's scores behind any
            # same-pair projection filler writing another chunk of the tile
            qTc = [[work.tile([P, 512], BF16, tag=f"qT{p}{c}", name=f"qT{p}{c}")
                    for c in range(4)] for p in range(2)]
            kTc = [[work.tile([P, 512], BF16, tag=f"kT{p}{c}", name=f"kT{p}{c}")
                    for c in range(4)] for p in range(2)]
            # vaug / outT split into small tiles: Tile hazard tracking is
            # per-tile, shared big tiles create false deps between writers
            # and concurrent readers. vaug is per-PAIR so pair-1's V
            # projection can run as fillers in later blocks.
            vaug = [[work.tile([P, 2, 80], BF16, tag=f"va{p}{ct}",
                               name=f"va{p}{ct}") for ct in range(CT)]
                    for p in range(2)]
            for p in range(2):
                for ct in range(CT):
                    nc.vector.memset(vaug[p][ct][:, :, SPH:SPH + 1], 1.0)
            # outT[qc4][p][qsub]: [pair-head rows (a*64+s), 128 q cols]
            outT = [[[work.tile([P, P], BF16, tag=f"oT{qc4}{p}{qs}",
                                name=f"oT{qc4}{p}{qs}") for qs in range(4)]
                     for p in range(2)]
                    for qc4 in range(4)]

            # ---- filler generators: each yield = ~2 PE matmuls ----------
            def gen_proj(which, p, qc4, csplit=None):
                """q/k projection chunk -> qTc/kTc[p][qc4].
                csplit=(lo,hi) restricts to a column sub-range (lead-in)."""
                src_sb = wk_sb if which == "k" else wq_sb
                x_sb = ct_sb if which == "k" else qt_sb
                dst = (kTc if which == "k" else qTc)[p][qc4]
                lo, hi = csplit if csplit else (512 * qc4, 512 * (qc4 + 1))
                w = hi - lo
                ps = psW.tile([P, w], F32, tag="W", name=f"ps{which}{p}{qc4}")
                for dc in range(DC):
                    nc.tensor.matmul(
                        ps[:, :],
                        src_sb[:, dc, P * p:P * (p + 1)],
                        x_sb[:, dc, lo:hi],
                        start=(dc == 0), stop=(dc == DC - 1),
                    )
                    if dc % 2 == 1 and dc != DC - 1:
                        yield
                # copy on DVE, not ACT: an ACT copy interleaved between exp's
                # would delay the exp stream that paces the attention loop
                nc.vector.tensor_copy(dst[:, lo - 512 * qc4:hi - 512 * qc4],
                                      ps[:, :])
                yield

            def gen_vproj(p, ct):
                """V projection (one pair's 2 heads) for one c-tile."""
                psv = psW.tile([P, 2 * SPH], F32, tag="W", name=f"psv{p}{ct}")
                for dc in range(DC):
                    nc.tensor.matmul(
                        psv[:, :],
                        ct_sb[:, dc, P * ct:P * (ct + 1)],
                        wv_sb[:, dc, P * p:P * (p + 1)],
                        start=(dc == 0), stop=(dc == DC - 1),
                    )
                nc.vector.tensor_copy(
                    vaug[p][ct][:, :, 0:SPH],
                    psv[:, :].rearrange("p (h s) -> p h s", h=2),
                )
                yield

            def gen_wo(qc4):
                """Output projection for q-chunk qc4: per (qt4, dh) one
                [128,512] accumulator over the 2 pairs."""
                for qt4 in range(4):
                    qt = 4 * qc4 + qt4
                    osb = outp.tile([P, D], BF16, tag="osb", name="osb")
                    for dh in range(2):
                        wx = psW.tile([P, 512], F32, tag="W", name="wx")
                        for p2 in range(2):
                            nc.tensor.matmul(
                                wx[:, :],
                                outT[qc4][p2][qt4][:, :],
                                wo_sb[:, p2, 512 * dh:512 * (dh + 1)],
                                start=(p2 == 0), stop=(p2 == 1))
                        nc.vector.tensor_copy(osb[:, 512 * dh:512 * (dh + 1)], wx)
                        if dh == 0:
                            yield
                    # DMA emitted before the final yield: a generator pumped an
                    # exact number of steps never runs code after its last yield
                    eng = nc.gpsimd if qt4 % 2 == 0 else nc.sync
                    eng.dma_start(out=out_d[P * qt:P * (qt + 1), :], in_=osb)
                    yield

            # ---- epilogue: normalize po rows by the denominator column,
            # transpose into outT. 7 steps, software-pipelined: step k does
            # the ts-muls for qsub k and the transposes/outT copy for qsub
            # k-1, so the PE transposes always consume slot-old DVE results
            # instead of stalling on same-slot ts-muls.
            def gen_epi(poAB, qc4, p):
                stg, rc = [], []
                for a in range(2):
                    # stage po to SBUF eagerly so the po bank recycles before
                    # the next block's first PV accumulation
                    s = epi.tile([P, 4, 65], F32, tag="stg", name="stg")
                    nc.vector.tensor_copy(s, poAB[a][:, :, :])
                    r = epi.tile([P, 4], F32, tag="rc", name="rc")
                    nc.vector.reciprocal(r, s[:, :, SPH:SPH + 1])
                    stg.append(s)
                    rc.append(r)
                    yield
                pns = [None] * 4
                for k in range(5):
                    if k < 4:
                        pq = []
                        for a in range(2):
                            pn = epi.tile([P, SPH], BF16, tag="pn", name="pn")
                            nc.vector.tensor_scalar_mul(
                                pn, stg[a][:, k, 0:SPH], rc[a][:, k:k + 1])
                            pq.append(pn)
                        pns[k] = pq
                    if k >= 1:
                        qs = k - 1
                        trp = psW.tile([P, P], BF16, tag="W", name="trp")
                        for a in range(2):
                            nc.tensor.matmul(
                                trp[SPH * a:SPH * (a + 1), :], pns[qs][a],
                                id_sb, is_transpose=True)
                        nc.vector.tensor_copy(outT[qc4][p][qs][:, :], trp)
                    yield

            # ---- attention block: two heads of a pair interleaved -------
            # Heads a=0/1 live at partition bases 0/64 in qTp/kTp; scores for
            # both go into one [128,2,512] PSUM tile so a single exp covers
            # both heads. slots: per-ct filler generators (~2 steps each).
            def attn_block(qc4, p, slots):
                q0 = 512 * qc4
                if NOFILL:
                    seen = []
                    for gens in slots.values():
                        for gen, _ in gens:
                            if all(gen is not g for g in seen):
                                seen.append(gen)
                    for gen in seen:
                        for _ in gen:
                            pass
                poAB = [psA.tile([P, 4, 65], F32, tag=f"A{a}", name=f"po{a}")
                        for a in range(2)]
                pend = None
                for ct in range(CT + 1):
                    if ct < CT:
                        Sp = psS.tile([P, 2, 512], F32, tag="S", name="Sp")
                        for a in range(2):
                            lo, hi = SPH * a, SPH * (a + 1)
                            nc.tensor.matmul(
                                Sp[:, a, :],
                                kTc[p][ct // 4][lo:hi,
                                                P * (ct % 4):P * (ct % 4 + 1)],
                                qTc[p][qc4][lo:hi, :],
                                start=True, stop=True)
                        if masked:
                            mk = ptp.tile([P, 512], BF16, tag="mk", name="mk")
                            nc.sync.dma_start(
                                out=mk,
                                in_=mk_d[P * ct:P * (ct + 1), q0:q0 + 512])
                            for a in range(2):
                                nc.vector.tensor_add(Sp[:, a, :], Sp[:, a, :], mk)
                    if ct >= 1:
                        pct = ct - 1
                        PT = ptp.tile([P, 2, 512], BF16, tag="PT", name="PT")
                        nc.scalar.activation(
                            PT[:, :, :], pend[:, :, :],
                            mybir.ActivationFunctionType.Exp)
                    if ct < CT:
                        pend = Sp
                        if not NOFILL:
                            for gen, nsteps in slots.get(ct, ()):
                                for _ in range(nsteps):
                                    try:
                                        next(gen)
                                    except StopIteration:
                                        break
                    if ct >= 1:
                        # flipped PV: PT slices stationary, vaug moving.
                        # One accumulation group per po BANK: start only on
                        # the bank's first matmul, stop only on its last.
                        # Emitted AFTER the fillers: PE reaches these just as
                        # exp(ct-1) completes -- emitting them earlier parks
                        # them in the 4-deep wait queue, which blocks the
                        # younger fillers/scores and adds ~140ns per ct.
                        for a in range(2):
                            for qs in range(4):
                                nc.tensor.matmul(
                                    poAB[a][:, qs, :],
                                    PT[:, a, P * qs:P * (qs + 1)],
                                    vaug[p][pct][:, a, 0:SPH + 1],
                                    start=(pct == 0 and qs == 0),
                                    stop=(pct == CT - 1 and qs == 3))
                return poAB

            def drain(gen):
                for _ in gen:
                    pass

            # ---- schedule: all pair-0 blocks, then all pair-1 blocks ----
            # After the projection-heavy first block, the ACT exp stream
            # (133us total) is the kernel's floor; pair-0-first keeps every
            # later block ACT-paced with evenly spread PE fillers.

            # lead-in: race the input DMA (k/q chunk 0 in arrival order)
            drain(gen_proj("k", 0, 0, csplit=(0, 256)))
            drain(gen_proj("q", 0, 0, csplit=(0, 256)))
            drain(gen_proj("q", 0, 0, csplit=(256, 512)))
            drain(gen_proj("k", 0, 0, csplit=(256, 512)))

            # block (0,0): pair-0 V projections (1 step per ct, just in
            # time for PV) + remaining kTp[0] chunks racing the ctx DMA.
            vg0 = {ct: gen_vproj(0, ct) for ct in range(CT)}
            vg1 = {ct: gen_vproj(1, ct) for ct in range(CT)}
            g_k01 = gen_proj("k", 0, 1)
            g_k02 = gen_proj("k", 0, 2)
            g_k03 = gen_proj("k", 0, 3)
            g_q01 = gen_proj("q", 0, 1)
            po00 = attn_block(0, 0, {
                0: [(vg0[0], 1)], 1: [(vg0[1], 1), (g_k01, 2)],
                2: [(vg0[2], 1), (g_k01, 2)], 3: [(vg0[3], 1)],
                4: [(vg0[4], 1)], 5: [(vg0[5], 1), (g_k02, 2)],
                6: [(vg0[6], 1), (g_k02, 2)], 7: [(vg0[7], 1)],
                8: [(vg0[8], 1)], 9: [(vg0[9], 1), (g_k03, 2)],
                10: [(vg0[10], 1), (g_k03, 2)], 11: [(vg0[11], 1)],
                12: [(vg0[12], 1), (g_q01, 2)],
                13: [(vg0[13], 1), (g_q01, 2)],
                14: [(vg0[14], 1)], 15: [(vg0[15], 1)],
            })

            # pair-0 blocks carry pair-1's V projection and the q1x chunks
            # (opposite-pair tiles: no hazard against their own scores)
            # one filler step (~427ns) per slot: a 2-step slot overflows the
            # per-ct ACT budget and each overflow costs a psS-ring ping-pong
            g_epi = gen_epi(po00, 0, 0)
            g_q02 = gen_proj("q", 0, 2)
            g_q11 = gen_proj("q", 1, 1)
            # epilogue qs-steps spread one per slot (0-4): each transpose's
            # psW slot frees before the next psW-using filler allocates
            po10 = attn_block(1, 0, {
                0: [(g_epi, 2)], 1: [(g_epi, 1)], 2: [(g_epi, 1), (vg1[0], 1)],
                3: [(g_epi, 1), (g_q02, 1)], 4: [(g_epi, 1), (vg1[1], 1)],
                5: [(g_epi, 1), (g_q02, 1)],
                6: [(vg1[2], 1)], 7: [(g_q02, 1)], 8: [(vg1[3], 1)],
                9: [(g_q02, 1)], 10: [(g_q11, 1)], 11: [(vg1[4], 1)],
                12: [(g_q11, 1)], 13: [(g_q11, 1)], 14: [(g_q11, 1)],
                15: [(vg1[5], 1)],
            })

            g_epi = gen_epi(po10, 1, 0)
            g_q03 = gen_proj("q", 0, 3)
            g_q12 = gen_proj("q", 1, 2)
            po20 = attn_block(2, 0, {
                0: [(g_epi, 2)], 1: [(g_epi, 1)], 2: [(g_epi, 1), (vg1[6], 1)],
                3: [(g_epi, 1), (g_q03, 1)], 4: [(g_epi, 1), (vg1[7], 1)],
                5: [(g_epi, 1), (g_q03, 1)],
                6: [(vg1[8], 1)], 7: [(g_q03, 1)], 8: [(vg1[9], 1)],
                9: [(g_q03, 1)], 10: [(g_q12, 1)], 11: [(vg1[10], 1)],
                12: [(g_q12, 1)], 13: [(g_q12, 1)], 14: [(g_q12, 1)],
            })

            g_epi = gen_epi(po20, 2, 0)
            g_q10 = gen_proj("q", 1, 0)
            g_k10 = gen_proj("k", 1, 0)
            po30 = attn_block(3, 0, {
                0: [(g_epi, 2)], 1: [(g_epi, 1)], 2: [(g_epi, 1), (vg1[11], 1)],
                3: [(g_epi, 1), (g_q10, 1)], 4: [(g_epi, 1), (vg1[12], 1)],
                5: [(g_epi, 1), (g_q10, 1)],
                6: [(vg1[13], 1)], 7: [(g_q10, 1)], 8: [(vg1[14], 1)],
                9: [(g_q10, 1)], 10: [(g_k10, 1)], 11: [(vg1[15], 1)],
                12: [(g_k10, 1)], 13: [(g_k10, 1)], 14: [(g_k10, 1)],
            })

            # pair-1 blocks: k1x chunks (same-pair, subtile deps), then one
            # wo per block chasing its own epilogue. k11 must land by slot 2
            # (scores(4) read it), so it alone gets 2-step slots.
            g_epi = gen_epi(po30, 3, 0)
            g_k11 = gen_proj("k", 1, 1)
            g_k12 = gen_proj("k", 1, 2)
            g_k13 = gen_proj("k", 1, 3)
            po01 = attn_block(0, 1, {
                0: [(g_epi, 2), (g_k11, 2)],
                1: [(g_epi, 1), (g_k11, 2)], 2: [(g_epi, 1), (g_k12, 1)],
                3: [(g_epi, 1), (g_k12, 1)], 4: [(g_epi, 1), (g_k12, 1)],
                5: [(g_epi, 1), (g_k12, 1)],
                6: [(g_k13, 1)], 7: [(g_k13, 1)], 8: [(g_k13, 1)],
                9: [(g_k13, 1)],
            })

            g_epi = gen_epi(po01, 0, 1)
            g_wo0 = gen_wo(0)
            g_q13 = gen_proj("q", 1, 3)
            po11 = attn_block(1, 1, {
                0: [(g_epi, 2)], 1: [(g_epi, 1)], 2: [(g_epi, 1)],
                3: [(g_epi, 1), (g_q13, 1)], 4: [(g_epi, 1), (g_q13, 1)],
                5: [(g_epi, 1), (g_q13, 1)],
                6: [(g_q13, 1)],
                7: [(g_wo0, 1)], 8: [(g_wo0, 1)], 9: [(g_wo0, 1)],
                10: [(g_wo0, 1)], 11: [(g_wo0, 1)], 12: [(g_wo0, 1)],
                13: [(g_wo0, 1)], 14: [(g_wo0, 1)],
            })

            g_epi = gen_epi(po11, 1, 1)
            g_wo1 = gen_wo(1)
            po21 = attn_block(2, 1, {
                0: [(g_epi, 2)], 1: [(g_epi, 1)], 2: [(g_epi, 1)],
                3: [(g_epi, 1)], 4: [(g_epi, 1)], 5: [(g_epi, 1)],
                7: [(g_wo1, 1)], 8: [(g_wo1, 1)], 9: [(g_wo1, 1)],
                10: [(g_wo1, 1)], 11: [(g_wo1, 1)], 12: [(g_wo1, 1)],
                13: [(g_wo1, 1)], 14: [(g_wo1, 1)],
            })

            g_epi = gen_epi(po21, 2, 1)
            g_wo2 = gen_wo(2)
            po31 = attn_block(3, 1, {
                0: [(g_epi, 2)], 1: [(g_epi, 1)], 2: [(g_epi, 1)],
                3: [(g_epi, 1)], 4: [(g_epi, 1)], 5: [(g_epi, 1)],
                7: [(g_wo2, 1)], 8: [(g_wo2, 1)], 9: [(g_wo2, 1)],
                10: [(g_wo2, 1)], 11: [(g_wo2, 1)], 12: [(g_wo2, 1)],
                13: [(g_wo2, 1)], 14: [(g_wo2, 1)],
            })

            # ---- tail: last epilogue + wo(3) ----------------------------
            # Latency-bound chain off the last exp: read po from PSUM
            # directly (no staging), run all four qsubs' normalize/transpose
            # chains first, then the wo matmuls + copies (DVE/ACT split) +
            # out-DMA halves as each half lands.
            rc = []
            for a in range(2):
                r = epi.tile([P, 4], F32, tag="rc", name=f"trc{a}")
                nc.vector.reciprocal(r, po31[a][:, :, SPH:SPH + 1])
                rc.append(r)
            for qs in range(4):
                trp = psW.tile([P, P], BF16, tag="W", name="trp")
                for a in range(2):
                    pn = epi.tile([P, SPH], BF16, tag="pn", name="pn")
                    # a=0 normalize on DVE, a=1 on ACT: halves the serial
                    # epilogue chain feeding the transposes
                    if a == 0:
                        nc.vector.tensor_scalar_mul(
                            pn, po31[a][:, qs, 0:SPH], rc[a][:, qs:qs + 1])
                    else:
                        nc.scalar.mul(pn, po31[a][:, qs, 0:SPH],
                                      rc[a][:, qs:qs + 1])
                    nc.tensor.matmul(
                        trp[SPH * a:SPH * (a + 1), :], pn, id_sb,
                        is_transpose=True)
                # outT copies split ACT/DVE
                if qs % 2 == 0:
                    nc.scalar.copy(outT[3][1][qs][:, :], trp)
                else:
                    nc.vector.tensor_copy(outT[3][1][qs][:, :], trp)
            osbt = [outp.tile([P, D], BF16, tag="osb", name=f"osbt{q}")
                    for q in range(4)]
            # each qs gets its own PSUM banks (psS slots, the dead po banks,
            # and the psW ring) so no wo matmul waits on an earlier qs's
            # copies. All pair-0 (start) matmuls are emitted first: the
            # pair-1 (stop) matmuls park in the 4-deep wait queue and would
            # otherwise block younger pair-0 work. qs3 takes the po banks
            # (drained by the ts-muls) -- earlier than the psW ring, which
            # still cycles the transposes.
            wxq = [
                psS.tile([P, 2, 512], F32, tag="S", name="twx0"),
                psS.tile([P, 2, 512], F32, tag="S", name="twx1"),
                [psW.tile([P, 512], F32, tag="W", name="twx2a"),
                 psW.tile([P, 512], F32, tag="W", name="twx2b")],
                [psA.tile([P, 512], F32, tag="A0", name="twx3a"),
                 psA.tile([P, 512], F32, tag="A1", name="twx3b")],
            ]

            def wxv(qs, dh):
                return wxq[qs][:, dh, :] if qs < 2 else wxq[qs][dh][:, :]

            def wo_p0(qs):
                for dh in range(2):
                    nc.tensor.matmul(
                        wxv(qs, dh), outT[3][0][qs][:, :],
                        wo_sb[:, 0, 512 * dh:512 * (dh + 1)],
                        start=True, stop=False, skip_group_check=True)

            def wo_p1(qs):
                qt = 4 * 3 + qs
                for dh in range(2):
                    nc.tensor.matmul(
                        wxv(qs, dh), outT[3][1][qs][:, :],
                        wo_sb[:, 1, 512 * dh:512 * (dh + 1)],
                        start=False, stop=True, skip_group_check=True)
                    dst = osbt[qs][:, 512 * dh:512 * (dh + 1)]
                    # dh0 on ACT, dh1 on DVE: parallel drains
                    if dh == 0:
                        nc.scalar.copy(dst, wxv(qs, dh))
                    else:
                        nc.vector.tensor_copy(dst, wxv(qs, dh))
                # one full-tile DMA per qs, spread across the three DMA-
                # capable queues: the per-queue wait+issue pipeline (~1.2us
                # per DMA, serialized) would otherwise gate the last transfer
                eng = [nc.gpsimd, nc.scalar, nc.gpsimd, nc.sync][qs]
                eng.dma_start(out=out_d[P * qt:P * (qt + 1), :], in_=osbt[qs])

            # qs2/qs3's pair-0 matmuls wait on the po-bank / psW-ring drain;
            # emitting them before qs0/qs1's pair-1 work would fill the
            # 4-deep wait queue and block it
            wo_p0(0)
            wo_p0(1)
            wo_p1(0)
            wo_p1(1)
            wo_p0(3)
            wo_p0(2)
            wo_p1(2)
            wo_p1(3)

    nc.compile()
    return nc


def _get_nc(masked: bool):
    if masked not in _NC_CACHE:
        _NC_CACHE[masked] = _build(masked)
    return _NC_CACHE[masked]


def kernel(query, context, attention_mask, Wq, Wk, Wv, Wo, **_unused):
    query = np.asarray(query, dtype=np.float32)
    context = np.asarray(context, dtype=np.float32)
    attention_mask = np.asarray(attention_mask, dtype=np.float32)
    Wq = np.asarray(Wq, dtype=np.float32)
    Wk = np.asarray(Wk, dtype=np.float32)
    Wv = np.asarray(Wv, dtype=np.float32)
    Wo = np.asarray(Wo, dtype=np.float32)

    masked = bool(np.any(attention_mask))
    nc = _get_nc(masked)

    bf = ml_dtypes.bfloat16
    # fold the 1/sqrt(SPH) score scale into Wq
    wq_s = (Wq * (SPH ** -0.5)).astype(bf)
    wk_s = Wk.astype(bf)
    wv_s = Wv.astype(bf)
    wo_s = Wo.astype(bf)
    ident = np.eye(P, dtype=bf)

    qtT = [np.ascontiguousarray(query[b].T).astype(bf) for b in range(B)]
    ctT = [np.ascontiguousarray(context[b].T).astype(bf) for b in range(B)]
    if masked:
        mkT = [np.ascontiguousarray((attention_mask[b, 0] * NEG_INF).T).astype(bf)
               for b in range(B)]

    in_maps = []
    for c in range(8):
        b, g = c // 4, c % 4
        hs = slice(NH * g, NH * (g + 1))
        im = {
            "qt": qtT[b],
            "ctx": ctT[b],
            "wq": np.ascontiguousarray(wq_s[:, hs, :]).reshape(D, NH * SPH),
            "wk": np.ascontiguousarray(wk_s[:, hs, :]).reshape(D, NH * SPH),
            "wv": np.ascontiguousarray(wv_s[:, hs, :]).reshape(D, NH * SPH),
            "wo": np.ascontiguousarray(wo_s[hs]).reshape(NH * SPH, D),
            "ident": ident,
        }
        if masked:
            im["maskT"] = mkT[b]
        in_maps.append(im)

    global _last_in_maps
    _last_in_maps = in_maps
    res = run_bass_kernel_spmd(nc, in_maps, core_ids=list(range(8)))

    out = np.zeros((B, S, D), dtype=np.float32)
    for c in range(8):
        out[c // 4] += res.results[c]["out"].astype(np.float32)
    return out
